# revision 15
# baseline (speedup 1.0000x reference)
"""Supervised-contrastive point-cloud loss on 8 TRN2 NeuronCores.

Full inputs: features [8, 128, 4096] f32, labels_all [8, 4096] int.
Data-parallel: one cloud per core. Each core computes the SUM of per-point
losses for its cloud; the host averages (sum / N / B).

Math (per cloud, fmap [C=128, N=4096], labels [N], 16 classes):
  v = normalize(fmap.T)                 (rows unit-norm)
  E = exp(v @ v.T)                      (TEMP cancels in pos/(pos+neg))
  sel[i] = sum_{j: lab j == lab i} E[ij]   (incl. diagonal e)
  T[i]   = sum_j E[ij]
  A = sel - e ; B = T - sel ; n = count[lab_i] ; nbar = N - n
  loss_i = ln(A*nbar + B*n) - ln(A*nbar)
The NxN mask is never materialized: per-class sums come from a one-hot
[128,16] matmul against exp tiles, accumulated in PSUM over j-blocks.

Engine budget per core: ACT does the 16.8M exps (the wall), PE does the gram
+ class-sum matmuls, DVE does normalize/epilogue elementwise, GPSIMD does
partition reductions (norms, counts, final sum), DMA does partition
broadcasts. PSUM: rotating "work" pool ([128,1024] x 3 = 6 banks) for gram
tiles + transposed-class-sum accumulator, [16,1024] class-sum pool (2 banks).
Everything else is PSUM-free so matmuls keep <=2 sync waits (walrus limit).
"""

import numpy as np
from contextlib import ExitStack

import concourse.bass as bass
import concourse.bacc as bacc
import concourse.bass_isa as bass_isa
import concourse.tile as tile
from concourse import mybir
from concourse.bass_utils import run_bass_kernel_spmd

F32 = mybir.dt.float32
BF16 = mybir.dt.bfloat16
I32 = mybir.dt.int32
AF = mybir.ActivationFunctionType
ALU = mybir.AluOpType
AX = mybir.AxisListType

B = 8
C = 128
N = 4096
NB = N // 128          # 32 point blocks of 128
NCLS = 16
ISUP = 1024            # i-super width (columns of E per cs accumulator)
NSUP = N // ISUP       # 4
HW = 1024              # work-tile / exp chunk width (2 PSUM banks)
E_CONST = float(np.exp(1.0))


def _body(ctx: ExitStack, tc: "tile.TileContext", feat, lab, outp):
    nc = tc.nc

    const = ctx.enter_context(tc.tile_pool(name="const", bufs=1))
    sb = ctx.enter_context(tc.tile_pool(name="sb", bufs=1))
    e_pool = ctx.enter_context(tc.tile_pool(name="e", bufs=8))
    work = ctx.enter_context(tc.tile_pool(name="work", bufs=3, space="PSUM"))
    csp = ctx.enter_context(tc.tile_pool(name="csp", bufs=1, space="PSUM"))

    def wtile(shape, tag="work"):
        return work.tile(shape, F32, tag=tag, name="wk")

    # Preload the one ACT table set that serves every function we use
    # (natural_log_exp_and_others: exp, ln, copy, identity) so the bacc
    # fixpoint pass doesn't insert per-function loads mid-kernel.
    from concourse.hw_specs import get_activation_tables

    tables = list(get_activation_tables(nc.m.arch).keys())
    nle_id = tables.index("natural_log_exp_and_others")
    tl = mybir.InstLoadActFuncSet(
        name=nc.get_next_instruction_name(), act_func_set_id=nle_id, ins=[], outs=[]
    )
    nc.scalar.add_instruction(tl)

    # ---------------- constants ----------------
    iota_i = const.tile([128, NCLS], I32, tag="iota_i")
    nc.gpsimd.iota(iota_i, pattern=[[1, NCLS]], base=0, channel_multiplier=0)
    iota_f = const.tile([128, NCLS], F32, tag="iota_f")
    nc.vector.tensor_copy(iota_f, iota_i)

    pidx_i = const.tile([128, 1], I32, tag="pidx_i")
    nc.gpsimd.iota(pidx_i, pattern=[[1, 1]], base=0, channel_multiplier=1)
    pidx_f = const.tile([128, 1], F32, tag="pidx_f")
    nc.vector.tensor_copy(pidx_f, pidx_i)

    i128 = const.tile([128, 128], I32, tag="i128")
    nc.gpsimd.iota(i128, pattern=[[1, 128]], base=0, channel_multiplier=0)
    i128_f = const.tile([128, 128], F32, tag="i128_f")
    nc.vector.tensor_copy(i128_f, i128)
    ident128 = const.tile([128, 128], F32, tag="ident128")
    nc.vector.tensor_scalar(
        out=ident128, in0=i128_f, scalar1=pidx_f, scalar2=None, op0=ALU.is_equal
    )

    # ---------------- load + normalize features (chunk-pipelined) ----------
    # per 1024-col chunk: DMA -> vsq (DVE) -> ns partition-reduce (GPSIMD) ->
    # ln (ACT) -> rinv = exp(-0.5*ln) (ACT) -> partition-broadcast (DMA) ->
    # vn = v * rinv_bc (DVE, bf16 out). No PSUM anywhere: the main loop's
    # gram tiles own the work pool from t=0. (Rsqrt ACT table is banned.)
    v_sb = sb.tile([128, N], F32, tag="v_sb")
    vsq = sb.tile([128, N], F32, tag="vsq")
    ns_all = sb.tile([128, N], F32, tag="ns_all")
    lns = sb.tile([128, N], F32, tag="lns")
    rinv_bc = sb.tile([128, N], BF16, tag="rinv_bc")
    vn_bf = sb.tile([128, N], BF16, tag="vn_bf")
    bounds = [0, 512, 1024] + [(k + 1) * HW for k in range(1, N // HW)]
    for cl, ch in zip(bounds[:-1], bounds[1:]):
        nc.sync.dma_start(out=v_sb[:, cl:ch], in_=feat[:, cl:ch])
        nc.vector.tensor_mul(vsq[:, cl:ch], v_sb[:, cl:ch], v_sb[:, cl:ch])
        nc.gpsimd.partition_all_reduce(
            ns_all[:, cl:ch], vsq[:, cl:ch], channels=128,
            reduce_op=bass_isa.ReduceOp.add,
        )
        nc.scalar.activation(lns[:, cl:ch], ns_all[:, cl:ch], AF.Ln)
        nc.scalar.activation(rinv_bc[:, cl:ch], lns[:, cl:ch], AF.Exp, scale=-0.5)
        nc.vector.tensor_mul(vn_bf[:, cl:ch], v_sb[:, cl:ch], rinv_bc[:, cl:ch])

    # ---------------- labels -> one-hot + class counts (PSUM-free) ---------
    labels_sb = sb.tile([128, NB], F32, tag="labels_sb")
    nc.gpsimd.dma_start(out=labels_sb, in_=lab[:, :])

    oh_f = sb.tile([128, NB * NCLS], F32, tag="oh_f")  # [128, 512]
    for b in range(NB):
        nc.vector.tensor_scalar(
            out=oh_f[:, b * NCLS : (b + 1) * NCLS],
            in0=iota_f,
            scalar1=labels_sb[:, b : b + 1],
            scalar2=None,
            op0=ALU.is_equal,
        )
    oh_b = sb.tile([128, NB * NCLS], BF16, tag="oh_b")
    nc.vector.tensor_copy(oh_b, oh_f)

    # counts[c] = #points of class c: partition all-reduce then fold blocks
    cnt_all = sb.tile([128, NB * NCLS], F32, tag="cnt_all")
    nc.gpsimd.partition_all_reduce(
        cnt_all, oh_f, channels=128, reduce_op=bass_isa.ReduceOp.add
    )
    n_bc = sb.tile([128, NCLS], F32, tag="n_bc")
    nc.vector.tensor_reduce(
        out=n_bc,
        in_=cnt_all.rearrange("p (b c) -> p c b", c=NCLS),
        axis=AX.X,
        op=ALU.add,
    )

    n_rep = sb.tile([128, NB * NCLS], F32, tag="n_rep")
    for b in range(NB):
        nc.vector.tensor_copy(n_rep[:, b * NCLS : (b + 1) * NCLS], n_bc)
    # n_row[p, b] = count[label of point 128*b+p]
    n_row = sb.tile([128, NB], F32, tag="n_row")
    nrm = sb.tile([128, NB * NCLS], F32, tag="nrm")
    nc.vector.tensor_mul(nrm, oh_f, n_rep)
    nc.vector.tensor_reduce(
        out=n_row,
        in_=nrm.rearrange("p (b c) -> p b c", c=NCLS),
        axis=AX.X,
        op=ALU.add,
    )

    # ---------------- main loop: gram -> exp -> class-sum ------------------
    # cst_ps accumulates the transposed class-sums; allocated at the first
    # super's epilogue, held to the end (work rotation drops 3 -> 2 slots).
    cst_ps = wtile([128, NB * NCLS])  # [128, 512], held to the end
    masked = sb.tile([128, NB * NCLS], F32, tag="masked")

    for s in range(NSUP):
        cs_ps = csp.tile([NCLS, ISUP], F32, tag="cs")
        for j in range(NB):
            for h in range(ISUP // HW):
                g = wtile([128, HW])
                for q in range(HW // 512):
                    col = s * ISUP + h * HW + q * 512
                    nc.tensor.matmul(
                        g[:, q * 512 : (q + 1) * 512],
                        lhsT=vn_bf[:, j * 128 : (j + 1) * 128],
                        rhs=vn_bf[:, col : col + 512],
                        start=True,
                        stop=True,
                    )
                e = e_pool.tile([128, HW], BF16, tag="e")
                nc.scalar.activation(e, g, AF.Exp)
                for q in range(HW // 512):
                    nc.tensor.matmul(
                        cs_ps[:, h * HW + q * 512 : h * HW + (q + 1) * 512],
                        lhsT=oh_b[:, j * NCLS : (j + 1) * NCLS],
                        rhs=e[:, q * 512 : (q + 1) * 512],
                        start=(j == 0),
                        stop=(j == NB - 1),
                    )
        # evacuate + transpose this super's class-sums while later supers run
        cs_sb = sb.tile([NCLS, ISUP], F32, tag=f"cs_sb{s}", name=f"cs_sb{s}")
        for hh in range(2):
            nc.vector.tensor_copy(
                cs_sb[:, hh * (ISUP // 2) : (hh + 1) * (ISUP // 2)],
                cs_ps[:, hh * (ISUP // 2) : (hh + 1) * (ISUP // 2)],
            )
        nblk = ISUP // 128
        for bb in range(nblk):
            b = s * nblk + bb
            nc.tensor.transpose(
                cst_ps[:, b * NCLS : (b + 1) * NCLS],
                in_=cs_sb[:, bb * 128 : (bb + 1) * 128],
                identity=ident128[0:NCLS, 0:NCLS],
            )
        nc.vector.tensor_mul(
            masked[:, s * nblk * NCLS : (s + 1) * nblk * NCLS],
            cst_ps[:, s * nblk * NCLS : (s + 1) * nblk * NCLS],
            oh_f[:, s * nblk * NCLS : (s + 1) * nblk * NCLS],
        )

    # ---------------- epilogue ----------------
    sel = sb.tile([128, NB], F32, tag="sel")
    nc.vector.tensor_reduce(
        out=sel,
        in_=masked.rearrange("p (b c) -> p b c", c=NCLS),
        axis=AX.X,
        op=ALU.add,
    )
    tot = sb.tile([128, NB], F32, tag="tot")
    nc.vector.tensor_reduce(
        out=tot,
        in_=cst_ps.rearrange("p (b c) -> p b c", c=NCLS),
        axis=AX.X,
        op=ALU.add,
    )

    a_t = sb.tile([128, NB], F32, tag="a_t")
    nc.vector.tensor_scalar_add(a_t, sel, -E_CONST)  # A = sel - e
    b_t = sb.tile([128, NB], F32, tag="b_t")
    nc.vector.tensor_sub(b_t, tot, sel)  # B = T - sel
    nbar = sb.tile([128, NB], F32, tag="nbar")
    nc.vector.tensor_scalar(
        out=nbar, in0=n_row, scalar1=-1.0, scalar2=float(N),
        op0=ALU.mult, op1=ALU.add,
    )
    num = sb.tile([128, NB], F32, tag="num")
    nc.vector.tensor_mul(num, a_t, nbar)
    den = sb.tile([128, NB], F32, tag="den")
    nc.vector.tensor_mul(den, b_t, n_row)
    nc.vector.tensor_add(den, den, num)

    l_den = sb.tile([128, NB], F32, tag="l_den")
    nc.scalar.activation(l_den, den, AF.Ln)
    l_num = sb.tile([128, NB], F32, tag="l_num")
    nc.scalar.activation(l_num, num, AF.Ln)
    lt = sb.tile([128, NB], F32, tag="lt")
    nc.vector.tensor_sub(lt, l_den, l_num)

    lp = sb.tile([128, 1], F32, tag="lp")
    nc.vector.tensor_reduce(out=lp, in_=lt, axis=AX.X, op=ALU.add)
    lp_all = sb.tile([128, 1], F32, tag="lp_all")
    nc.gpsimd.partition_all_reduce(
        lp_all, lp, channels=128, reduce_op=bass_isa.ReduceOp.add
    )
    nc.sync.dma_start(out=outp[:, :], in_=lp_all[0:1, :])


def build_nc():
    nc = bacc.Bacc()
    feat = nc.declare_dram_parameter("features", [C, N], F32, isOutput=False)
    lab = nc.declare_dram_parameter("labels", [128, NB], F32, isOutput=False)
    outp = nc.declare_dram_parameter("out", [1, 1], F32, isOutput=True)
    with tile.TileContext(nc) as tc:
        with ExitStack() as ctx:
            _body(ctx, tc, feat[:, :], lab[:, :], outp)
    nc.finalize()
    return nc


_NC_CACHE = None


def _get_nc():
    global _NC_CACHE
    if _NC_CACHE is None:
        _NC_CACHE = build_nc()
    return _NC_CACHE


def make_in_maps(features: np.ndarray, labels_all: np.ndarray):
    in_maps = []
    for i in range(B):
        f = np.ascontiguousarray(features[i], dtype=np.float32)
        # labels_sb[p, b] = labels[128*b + p]
        l = np.ascontiguousarray(
            labels_all[i].astype(np.float32).reshape(NB, 128).T
        )
        in_maps.append({"features": f, "labels": l})
    return in_maps


def kernel(features: np.ndarray, labels_all: np.ndarray) -> np.ndarray:
    nc = _get_nc()
    in_maps = make_in_maps(features, labels_all)
    r = run_bass_kernel_spmd(nc, in_maps, core_ids=list(range(B)))
    sums = np.array([r.results[i]["out"][0, 0] for i in range(B)], dtype=np.float64)
    return np.float32(np.mean(sums) / N)


# revision 20
# speedup vs baseline: 1.0244x; 1.0244x over previous
"""Supervised-contrastive point-cloud loss on 8 TRN2 NeuronCores.

Full inputs: features [8, 128, 4096] f32, labels_all [8, 4096] int.
Data-parallel: one cloud per core. Each core computes the SUM of per-point
losses for its cloud; the host averages (sum / N / B).

Math (per cloud, fmap [C=128, N=4096], labels [N], 16 classes):
  v = normalize(fmap.T)                 (rows unit-norm)
  E = exp(v @ v.T)                      (TEMP cancels in pos/(pos+neg))
  sel[i] = sum_{j: lab j == lab i} E[ij]   (incl. diagonal e)
  T[i]   = sum_j E[ij]
  A = sel - e ; B = T - sel ; n = count[lab_i] ; nbar = N - n
  loss_i = ln(A*nbar + B*n) - ln(A*nbar)
The NxN mask is never materialized: per-class sums come from a one-hot
[128,16] matmul against exp tiles, accumulated in PSUM over j-blocks.

Engine budget per core: ACT does the 16.8M exps (the wall), PE does the gram
+ class-sum matmuls, DVE does normalize/epilogue elementwise, GPSIMD does
partition reductions (norms, counts, final sum), DMA does partition
broadcasts. PSUM: rotating "work" pool ([128,1024] x 3 = 6 banks) for gram
tiles + transposed-class-sum accumulator, [16,1024] class-sum pool (2 banks).
Everything else is PSUM-free so matmuls keep <=2 sync waits (walrus limit).
"""

import numpy as np
from contextlib import ExitStack

import concourse.bass as bass
import concourse.bacc as bacc
import concourse.bass_isa as bass_isa
import concourse.tile as tile
from concourse import mybir
from concourse.bass_utils import run_bass_kernel_spmd

F32 = mybir.dt.float32
BF16 = mybir.dt.bfloat16
I32 = mybir.dt.int32
AF = mybir.ActivationFunctionType
ALU = mybir.AluOpType
AX = mybir.AxisListType

B = 8
C = 128
N = 4096
NB = N // 128          # 32 point blocks of 128
NCLS = 16
ISUP = 1024            # i-super width (columns of E per cs accumulator)
NSUP = N // ISUP       # 4
HW = 1024              # work-tile / exp chunk width (2 PSUM banks)
E_CONST = float(np.exp(1.0))


def _body(ctx: ExitStack, tc: "tile.TileContext", feat, lab, outp):
    nc = tc.nc

    const = ctx.enter_context(tc.tile_pool(name="const", bufs=1))
    sb = ctx.enter_context(tc.tile_pool(name="sb", bufs=1))
    e_pool = ctx.enter_context(tc.tile_pool(name="e", bufs=8))
    work = ctx.enter_context(tc.tile_pool(name="work", bufs=3, space="PSUM"))
    csp = ctx.enter_context(tc.tile_pool(name="csp", bufs=2, space="PSUM"))

    def wtile(shape, tag="work"):
        return work.tile(shape, F32, tag=tag, name="wk")

    # Preload the one ACT table set that serves every function we use
    # (natural_log_exp_and_others: exp, ln, copy, identity) so the bacc
    # fixpoint pass doesn't insert per-function loads mid-kernel.
    from concourse.hw_specs import get_activation_tables

    tables = list(get_activation_tables(nc.m.arch).keys())
    nle_id = tables.index("natural_log_exp_and_others")
    tl = mybir.InstLoadActFuncSet(
        name=nc.get_next_instruction_name(), act_func_set_id=nle_id, ins=[], outs=[]
    )
    nc.scalar.add_instruction(tl)

    # ---------------- load + normalize features (chunk-pipelined) ----------
    # per 1024-col chunk: DMA -> vsq (DVE) -> ns partition-reduce (GPSIMD) ->
    # ln (ACT) -> rinv = exp(-0.5*ln) (ACT) -> partition-broadcast (DMA) ->
    # vn = v * rinv_bc (DVE, bf16 out). No PSUM anywhere: the main loop's
    # gram tiles own the work pool from t=0. (Rsqrt ACT table is banned.)
    v_sb = sb.tile([128, N], F32, tag="v_sb")
    vsq = sb.tile([128, N], F32, tag="vsq")
    ns_all = sb.tile([128, N], F32, tag="ns_all")
    lns = sb.tile([128, N], F32, tag="lns")
    rinv_bc = sb.tile([128, N], BF16, tag="rinv_bc")
    vn_bf = sb.tile([128, N], BF16, tag="vn_bf")
    bounds = [0, 512, 1024] + [(k + 1) * HW for k in range(1, N // HW)]
    for cl, ch in zip(bounds[:-1], bounds[1:]):
        nc.sync.dma_start(out=v_sb[:, cl:ch], in_=feat[:, cl:ch])
        nc.vector.tensor_mul(vsq[:, cl:ch], v_sb[:, cl:ch], v_sb[:, cl:ch])
        nc.gpsimd.partition_all_reduce(
            ns_all[:, cl:ch], vsq[:, cl:ch], channels=128,
            reduce_op=bass_isa.ReduceOp.add,
        )
        nc.scalar.activation(lns[:, cl:ch], ns_all[:, cl:ch], AF.Ln)
        nc.scalar.activation(rinv_bc[:, cl:ch], lns[:, cl:ch], AF.Exp, scale=-0.5)
        nc.vector.tensor_mul(vn_bf[:, cl:ch], v_sb[:, cl:ch], rinv_bc[:, cl:ch])

    # ---------------- constants ----------------
    iota_i = const.tile([128, NCLS], I32, tag="iota_i")
    nc.gpsimd.iota(iota_i, pattern=[[1, NCLS]], base=0, channel_multiplier=0)
    iota_f = const.tile([128, NCLS], F32, tag="iota_f")
    nc.vector.tensor_copy(iota_f, iota_i)

    pidx_i = const.tile([128, 1], I32, tag="pidx_i")
    nc.gpsimd.iota(pidx_i, pattern=[[1, 1]], base=0, channel_multiplier=1)
    pidx_f = const.tile([128, 1], F32, tag="pidx_f")
    nc.vector.tensor_copy(pidx_f, pidx_i)

    i128 = const.tile([128, 128], I32, tag="i128")
    nc.gpsimd.iota(i128, pattern=[[1, 128]], base=0, channel_multiplier=0)
    i128_f = const.tile([128, 128], F32, tag="i128_f")
    nc.vector.tensor_copy(i128_f, i128)
    ident128 = const.tile([128, 128], F32, tag="ident128")
    nc.vector.tensor_scalar(
        out=ident128, in0=i128_f, scalar1=pidx_f, scalar2=None, op0=ALU.is_equal
    )

    # ---------------- labels -> one-hot + class counts (PSUM-free) ---------
    labels_sb = sb.tile([128, NB], F32, tag="labels_sb")
    nc.gpsimd.dma_start(out=labels_sb, in_=lab[:, :])

    oh_f = sb.tile([128, NB * NCLS], F32, tag="oh_f")  # [128, 512]
    for b in range(NB):
        nc.vector.tensor_scalar(
            out=oh_f[:, b * NCLS : (b + 1) * NCLS],
            in0=iota_f,
            scalar1=labels_sb[:, b : b + 1],
            scalar2=None,
            op0=ALU.is_equal,
        )
    oh_b = sb.tile([128, NB * NCLS], BF16, tag="oh_b")
    nc.vector.tensor_copy(oh_b, oh_f)

    # counts[c] = #points of class c: partition all-reduce then fold blocks
    cnt_all = sb.tile([128, NB * NCLS], F32, tag="cnt_all")
    nc.gpsimd.partition_all_reduce(
        cnt_all, oh_f, channels=128, reduce_op=bass_isa.ReduceOp.add
    )
    n_bc = sb.tile([128, NCLS], F32, tag="n_bc")
    nc.vector.tensor_reduce(
        out=n_bc,
        in_=cnt_all.rearrange("p (b c) -> p c b", c=NCLS),
        axis=AX.X,
        op=ALU.add,
    )

    n_rep = sb.tile([128, NB * NCLS], F32, tag="n_rep")
    for b in range(NB):
        nc.vector.tensor_copy(n_rep[:, b * NCLS : (b + 1) * NCLS], n_bc)
    # n_row[p, b] = count[label of point 128*b+p]
    n_row = sb.tile([128, NB], F32, tag="n_row")
    nrm = sb.tile([128, NB * NCLS], F32, tag="nrm")
    nc.vector.tensor_mul(nrm, oh_f, n_rep)
    nc.vector.tensor_reduce(
        out=n_row,
        in_=nrm.rearrange("p (b c) -> p b c", c=NCLS),
        axis=AX.X,
        op=ALU.add,
    )

    # ---------------- main loop: gram -> exp -> class-sum ------------------
    # cst_ps accumulates the transposed class-sums; allocated at the first
    # super's epilogue, held to the end (work rotation drops 3 -> 2 slots).
    cst_ps = wtile([128, NB * NCLS])  # [128, 512], held to the end
    masked = sb.tile([128, NB * NCLS], F32, tag="masked")

    def emit_super_epilogue(s, cs_half):
        # evacuate + transpose a finished super's class-sums; emitted a few
        # j-iterations into the NEXT super so the PE's in-order queue doesn't
        # stall its gram stream on the DVE evacuation copies
        cs_sb = sb.tile([NCLS, ISUP], F32, tag=f"cs_sb{s}", name=f"cs_sb{s}")
        nblk = ISUP // 128
        for hh in range(2):
            nc.vector.tensor_copy(
                cs_sb[:, hh * (ISUP // 2) : (hh + 1) * (ISUP // 2)],
                cs_half[hh],
            )
            for bb in range(hh * nblk // 2, (hh + 1) * nblk // 2):
                b = s * nblk + bb
                nc.tensor.transpose(
                    cst_ps[:, b * NCLS : (b + 1) * NCLS],
                    in_=cs_sb[:, bb * 128 : (bb + 1) * 128],
                    identity=ident128[0:NCLS, 0:NCLS],
                )
        nc.vector.tensor_mul(
            masked[:, s * nblk * NCLS : (s + 1) * nblk * NCLS],
            cst_ps[:, s * nblk * NCLS : (s + 1) * nblk * NCLS],
            oh_f[:, s * nblk * NCLS : (s + 1) * nblk * NCLS],
        )

    pending = None  # (s, cs_half) of the previous super, epilogue not emitted
    for s in range(NSUP):
        # two 1-bank accumulators: the next super's WAR waits only its half's
        # evacuation copy instead of the full 2-bank one
        cs_half = [
            csp.tile([NCLS, ISUP // 2], F32, tag="cs", name=f"cs{s}_{q}")
            for q in range(2)
        ]
        for j in range(NB):
            if j == 8 and pending is not None:
                emit_super_epilogue(*pending)
                pending = None
            for h in range(ISUP // HW):
                g = wtile([128, HW])
                for q in range(HW // 512):
                    col = s * ISUP + h * HW + q * 512
                    nc.tensor.matmul(
                        g[:, q * 512 : (q + 1) * 512],
                        lhsT=vn_bf[:, j * 128 : (j + 1) * 128],
                        rhs=vn_bf[:, col : col + 512],
                        start=True,
                        stop=True,
                    )
                e = e_pool.tile([128, HW], BF16, tag="e")
                nc.scalar.activation(e, g, AF.Exp)
                for q in range(HW // 512):
                    nc.tensor.matmul(
                        cs_half[q],
                        lhsT=oh_b[:, j * NCLS : (j + 1) * NCLS],
                        rhs=e[:, q * 512 : (q + 1) * 512],
                        start=(j == 0),
                        stop=(j == NB - 1),
                    )
        pending = (s, cs_half)
    emit_super_epilogue(*pending)

    # ---------------- epilogue ----------------
    sel = sb.tile([128, NB], F32, tag="sel")
    nc.vector.tensor_reduce(
        out=sel,
        in_=masked.rearrange("p (b c) -> p b c", c=NCLS),
        axis=AX.X,
        op=ALU.add,
    )
    tot = sb.tile([128, NB], F32, tag="tot")
    nc.vector.tensor_reduce(
        out=tot,
        in_=cst_ps.rearrange("p (b c) -> p b c", c=NCLS),
        axis=AX.X,
        op=ALU.add,
    )

    a_t = sb.tile([128, NB], F32, tag="a_t")
    nc.vector.tensor_scalar_add(a_t, sel, -E_CONST)  # A = sel - e
    b_t = sb.tile([128, NB], F32, tag="b_t")
    nc.vector.tensor_sub(b_t, tot, sel)  # B = T - sel
    nbar = sb.tile([128, NB], F32, tag="nbar")
    nc.vector.tensor_scalar(
        out=nbar, in0=n_row, scalar1=-1.0, scalar2=float(N),
        op0=ALU.mult, op1=ALU.add,
    )
    num = sb.tile([128, NB], F32, tag="num")
    nc.vector.tensor_mul(num, a_t, nbar)
    den = sb.tile([128, NB], F32, tag="den")
    nc.vector.tensor_mul(den, b_t, n_row)
    nc.vector.tensor_add(den, den, num)

    l_den = sb.tile([128, NB], F32, tag="l_den")
    nc.scalar.activation(l_den, den, AF.Ln)
    l_num = sb.tile([128, NB], F32, tag="l_num")
    nc.scalar.activation(l_num, num, AF.Ln)
    lt = sb.tile([128, NB], F32, tag="lt")
    nc.vector.tensor_sub(lt, l_den, l_num)

    lp = sb.tile([128, 1], F32, tag="lp")
    nc.vector.tensor_reduce(out=lp, in_=lt, axis=AX.X, op=ALU.add)
    lp_all = sb.tile([128, 1], F32, tag="lp_all")
    nc.gpsimd.partition_all_reduce(
        lp_all, lp, channels=128, reduce_op=bass_isa.ReduceOp.add
    )
    nc.sync.dma_start(out=outp[:, :], in_=lp_all[0:1, :])


def build_nc():
    nc = bacc.Bacc()
    feat = nc.declare_dram_parameter("features", [C, N], F32, isOutput=False)
    lab = nc.declare_dram_parameter("labels", [128, NB], F32, isOutput=False)
    outp = nc.declare_dram_parameter("out", [1, 1], F32, isOutput=True)
    with tile.TileContext(nc) as tc:
        with ExitStack() as ctx:
            _body(ctx, tc, feat[:, :], lab[:, :], outp)
    nc.finalize()
    return nc


_NC_CACHE = None


def _get_nc():
    global _NC_CACHE
    if _NC_CACHE is None:
        _NC_CACHE = build_nc()
    return _NC_CACHE


def make_in_maps(features: np.ndarray, labels_all: np.ndarray):
    in_maps = []
    for i in range(B):
        f = np.ascontiguousarray(features[i], dtype=np.float32)
        # labels_sb[p, b] = labels[128*b + p]
        l = np.ascontiguousarray(
            labels_all[i].astype(np.float32).reshape(NB, 128).T
        )
        in_maps.append({"features": f, "labels": l})
    return in_maps


def kernel(features: np.ndarray, labels_all: np.ndarray) -> np.ndarray:
    nc = _get_nc()
    in_maps = make_in_maps(features, labels_all)
    r = run_bass_kernel_spmd(nc, in_maps, core_ids=list(range(B)))
    sums = np.array([r.results[i]["out"][0, 0] for i in range(B)], dtype=np.float64)
    return np.float32(np.mean(sums) / N)


# revision 22
# speedup vs baseline: 1.0327x; 1.0082x over previous
"""Supervised-contrastive point-cloud loss on 8 TRN2 NeuronCores.

Full inputs: features [8, 128, 4096] f32, labels_all [8, 4096] int.
Data-parallel: one cloud per core. Each core computes the SUM of per-point
losses for its cloud; the host averages (sum / N / B).

Math (per cloud, fmap [C=128, N=4096], labels [N], 16 classes):
  v = normalize(fmap.T)                 (rows unit-norm)
  E = exp(v @ v.T)                      (TEMP cancels in pos/(pos+neg))
  sel[i] = sum_{j: lab j == lab i} E[ij]   (incl. diagonal e)
  T[i]   = sum_j E[ij]
  A = sel - e ; B = T - sel ; n = count[lab_i] ; nbar = N - n
  loss_i = ln(A*nbar + B*n) - ln(A*nbar)
The NxN mask is never materialized: per-class sums come from a one-hot
[128,16] matmul against exp tiles, accumulated in PSUM over j-blocks.

Engine budget per core: ACT does the 16.8M exps (the wall), PE does the gram
+ class-sum matmuls, DVE does normalize/epilogue elementwise, GPSIMD does
partition reductions (norms, counts, final sum), DMA does partition
broadcasts. PSUM: rotating "work" pool ([128,1024] x 3 = 6 banks) for gram
tiles + the transposed-class-sum accumulator, plus two rotating [16,512]
class-sum accumulators (2 banks). Everything else is PSUM-free, which keeps
every matmul at <=2 sync waits (walrus MM limit; bacc moves extras onto
ldweights).
"""

import numpy as np
from contextlib import ExitStack

import concourse.bass as bass
import concourse.bacc as bacc
import concourse.bass_isa as bass_isa
import concourse.tile as tile
from concourse import mybir
from concourse.bass_utils import run_bass_kernel_spmd

F32 = mybir.dt.float32
BF16 = mybir.dt.bfloat16
I32 = mybir.dt.int32
AF = mybir.ActivationFunctionType
ALU = mybir.AluOpType
AX = mybir.AxisListType

B = 8
C = 128
N = 4096
NB = N // 128          # 32 point blocks of 128
NCLS = 16
ISUP = 1024            # i-super width (columns of E per cs accumulator)
NSUP = N // ISUP       # 4
HW = 1024              # work-tile / exp chunk width (2 PSUM banks)
E_CONST = float(np.exp(1.0))


def _body(ctx: ExitStack, tc: "tile.TileContext", feat, lab, outp):
    nc = tc.nc

    const = ctx.enter_context(tc.tile_pool(name="const", bufs=1))
    sb = ctx.enter_context(tc.tile_pool(name="sb", bufs=1))
    e_pool = ctx.enter_context(tc.tile_pool(name="e", bufs=8))
    work = ctx.enter_context(tc.tile_pool(name="work", bufs=3, space="PSUM"))
    csp = ctx.enter_context(tc.tile_pool(name="csp", bufs=2, space="PSUM"))

    def wtile(shape, tag="work"):
        return work.tile(shape, F32, tag=tag, name="wk")

    # Preload the one ACT table set that serves every function we use
    # (natural_log_exp_and_others: exp, ln, copy, identity) so the bacc
    # fixpoint pass doesn't insert per-function loads mid-kernel.
    from concourse.hw_specs import get_activation_tables

    tables = list(get_activation_tables(nc.m.arch).keys())
    nle_id = tables.index("natural_log_exp_and_others")
    tl = mybir.InstLoadActFuncSet(
        name=nc.get_next_instruction_name(), act_func_set_id=nle_id, ins=[], outs=[]
    )
    nc.scalar.add_instruction(tl)

    # ---------------- load + normalize features (chunk-pipelined) ----------
    # per 1024-col chunk: DMA -> vsq (DVE) -> ns partition-reduce (GPSIMD) ->
    # ln (ACT) -> rinv = exp(-0.5*ln) (ACT) -> partition-broadcast (DMA) ->
    # vn = v * rinv_bc (DVE, bf16 out). No PSUM anywhere: the main loop's
    # gram tiles own the work pool from t=0. (Rsqrt ACT table is banned.)
    v_sb = sb.tile([128, N], F32, tag="v_sb")
    vsq = sb.tile([128, N], F32, tag="vsq")
    ns_all = sb.tile([128, N], F32, tag="ns_all")
    lns = sb.tile([128, N], F32, tag="lns")
    rinv_bc = sb.tile([128, N], BF16, tag="rinv_bc")
    vn_bf = sb.tile([128, N], BF16, tag="vn_bf")
    bounds = [0, 512, 1024] + [(k + 1) * HW for k in range(1, N // HW)]
    for cl, ch in zip(bounds[:-1], bounds[1:]):
        nc.sync.dma_start(out=v_sb[:, cl:ch], in_=feat[:, cl:ch])
        nc.vector.tensor_mul(vsq[:, cl:ch], v_sb[:, cl:ch], v_sb[:, cl:ch])
        nc.gpsimd.partition_all_reduce(
            ns_all[:, cl:ch], vsq[:, cl:ch], channels=128,
            reduce_op=bass_isa.ReduceOp.add,
        )
        nc.scalar.activation(lns[:, cl:ch], ns_all[:, cl:ch], AF.Ln)
        nc.scalar.activation(rinv_bc[:, cl:ch], lns[:, cl:ch], AF.Exp, scale=-0.5)
        nc.vector.tensor_mul(vn_bf[:, cl:ch], v_sb[:, cl:ch], rinv_bc[:, cl:ch])

    # ---------------- constants ----------------
    iota_i = const.tile([128, NCLS], I32, tag="iota_i")
    nc.gpsimd.iota(iota_i, pattern=[[1, NCLS]], base=0, channel_multiplier=0)
    iota_f = const.tile([128, NCLS], F32, tag="iota_f")
    nc.vector.tensor_copy(iota_f, iota_i)

    pidx_i = const.tile([128, 1], I32, tag="pidx_i")
    nc.gpsimd.iota(pidx_i, pattern=[[1, 1]], base=0, channel_multiplier=1)
    pidx_f = const.tile([128, 1], F32, tag="pidx_f")
    nc.vector.tensor_copy(pidx_f, pidx_i)

    i128 = const.tile([128, 128], I32, tag="i128")
    nc.gpsimd.iota(i128, pattern=[[1, 128]], base=0, channel_multiplier=0)
    i128_f = const.tile([128, 128], F32, tag="i128_f")
    nc.vector.tensor_copy(i128_f, i128)
    ident128 = const.tile([128, 128], F32, tag="ident128")
    nc.vector.tensor_scalar(
        out=ident128, in0=i128_f, scalar1=pidx_f, scalar2=None, op0=ALU.is_equal
    )

    # ---------------- labels -> one-hot + class counts (PSUM-free) ---------
    labels_sb = sb.tile([128, NB], F32, tag="labels_sb")
    nc.gpsimd.dma_start(out=labels_sb, in_=lab[:, :])

    oh_f = sb.tile([128, NB * NCLS], F32, tag="oh_f")  # [128, 512]
    for b in range(NB):
        nc.vector.tensor_scalar(
            out=oh_f[:, b * NCLS : (b + 1) * NCLS],
            in0=iota_f,
            scalar1=labels_sb[:, b : b + 1],
            scalar2=None,
            op0=ALU.is_equal,
        )
    oh_b = sb.tile([128, NB * NCLS], BF16, tag="oh_b")
    nc.vector.tensor_copy(oh_b, oh_f)

    # counts[c] = #points of class c: partition all-reduce then fold blocks
    cnt_all = sb.tile([128, NB * NCLS], F32, tag="cnt_all")
    nc.gpsimd.partition_all_reduce(
        cnt_all, oh_f, channels=128, reduce_op=bass_isa.ReduceOp.add
    )
    n_bc = sb.tile([128, NCLS], F32, tag="n_bc")
    nc.vector.tensor_reduce(
        out=n_bc,
        in_=cnt_all.rearrange("p (b c) -> p c b", c=NCLS),
        axis=AX.X,
        op=ALU.add,
    )

    n_rep = sb.tile([128, NB * NCLS], F32, tag="n_rep")
    for b in range(NB):
        nc.vector.tensor_copy(n_rep[:, b * NCLS : (b + 1) * NCLS], n_bc)
    # n_row[p, b] = count[label of point 128*b+p]
    n_row = sb.tile([128, NB], F32, tag="n_row")
    nrm = sb.tile([128, NB * NCLS], F32, tag="nrm")
    nc.vector.tensor_mul(nrm, oh_f, n_rep)
    nc.vector.tensor_reduce(
        out=n_row,
        in_=nrm.rearrange("p (b c) -> p b c", c=NCLS),
        axis=AX.X,
        op=ALU.add,
    )

    # ---------------- main loop: gram -> exp -> class-sum ------------------
    # cst_ps accumulates the transposed class-sums; allocated at the first
    # super's epilogue, held to the end (work rotation drops 3 -> 2 slots).
    cst_ps = wtile([128, NB * NCLS])  # [128, 512], held to the end
    masked = sb.tile([128, NB * NCLS], F32, tag="masked")

    sel = sb.tile([128, NB], F32, tag="sel")
    tot = sb.tile([128, NB], F32, tag="tot")

    def emit_super_epilogue(s, cs_half):
        # evacuate + transpose a finished super's class-sums; emitted a few
        # j-iterations into the NEXT super so the PE's in-order queue doesn't
        # stall its gram stream on the DVE evacuation copies. The last super's
        # first copy runs on ACT (idle by then) in parallel with DVE.
        cs_sb = sb.tile([NCLS, ISUP], F32, tag=f"cs_sb{s}", name=f"cs_sb{s}")
        nblk = ISUP // 128
        for hh in range(2):
            dst = cs_sb[:, hh * (ISUP // 2) : (hh + 1) * (ISUP // 2)]
            if s == NSUP - 1 and hh == 0:
                nc.scalar.copy(dst, cs_half[hh])
            else:
                nc.vector.tensor_copy(dst, cs_half[hh])
            for bb in range(hh * nblk // 2, (hh + 1) * nblk // 2):
                b = s * nblk + bb
                nc.tensor.transpose(
                    cst_ps[:, b * NCLS : (b + 1) * NCLS],
                    in_=cs_sb[:, bb * 128 : (bb + 1) * 128],
                    identity=ident128[0:NCLS, 0:NCLS],
                )
        lo, hi = s * nblk * NCLS, (s + 1) * nblk * NCLS
        nc.vector.tensor_mul(
            masked[:, lo:hi], cst_ps[:, lo:hi], oh_f[:, lo:hi]
        )
        nc.vector.tensor_reduce(
            out=sel[:, s * nblk : (s + 1) * nblk],
            in_=masked[:, lo:hi].rearrange("p (b c) -> p b c", c=NCLS),
            axis=AX.X,
            op=ALU.add,
        )
        nc.vector.tensor_reduce(
            out=tot[:, s * nblk : (s + 1) * nblk],
            in_=cst_ps[:, lo:hi].rearrange("p (b c) -> p b c", c=NCLS),
            axis=AX.X,
            op=ALU.add,
        )

    pending = None  # (s, cs_half) of the previous super, epilogue not emitted
    for s in range(NSUP):
        # two 1-bank accumulators: the next super's WAR waits only its half's
        # evacuation copy instead of the full 2-bank one
        cs_half = [
            csp.tile([NCLS, ISUP // 2], F32, tag="cs", name=f"cs{s}_{q}")
            for q in range(2)
        ]
        for j in range(NB):
            if j == 8 and pending is not None:
                emit_super_epilogue(*pending)
                pending = None
            for h in range(ISUP // HW):
                g = wtile([128, HW])
                for q in range(HW // 512):
                    col = s * ISUP + h * HW + q * 512
                    nc.tensor.matmul(
                        g[:, q * 512 : (q + 1) * 512],
                        lhsT=vn_bf[:, j * 128 : (j + 1) * 128],
                        rhs=vn_bf[:, col : col + 512],
                        start=True,
                        stop=True,
                    )
                e = e_pool.tile([128, HW], BF16, tag="e")
                nc.scalar.activation(e, g, AF.Exp)
                for q in range(HW // 512):
                    nc.tensor.matmul(
                        cs_half[q],
                        lhsT=oh_b[:, j * NCLS : (j + 1) * NCLS],
                        rhs=e[:, q * 512 : (q + 1) * 512],
                        start=(j == 0),
                        stop=(j == NB - 1),
                    )
        pending = (s, cs_half)
    emit_super_epilogue(*pending)

    # ---------------- epilogue ----------------
    a_t = sb.tile([128, NB], F32, tag="a_t")
    nc.vector.tensor_scalar_add(a_t, sel, -E_CONST)  # A = sel - e
    b_t = sb.tile([128, NB], F32, tag="b_t")
    nc.vector.tensor_sub(b_t, tot, sel)  # B = T - sel
    nbar = sb.tile([128, NB], F32, tag="nbar")
    nc.vector.tensor_scalar(
        out=nbar, in0=n_row, scalar1=-1.0, scalar2=float(N),
        op0=ALU.mult, op1=ALU.add,
    )
    num = sb.tile([128, NB], F32, tag="num")
    nc.vector.tensor_mul(num, a_t, nbar)
    den = sb.tile([128, NB], F32, tag="den")
    nc.vector.tensor_mul(den, b_t, n_row)
    nc.vector.tensor_add(den, den, num)

    l_den = sb.tile([128, NB], F32, tag="l_den")
    nc.scalar.activation(l_den, den, AF.Ln)
    l_num = sb.tile([128, NB], F32, tag="l_num")
    nc.scalar.activation(l_num, num, AF.Ln)
    lt = sb.tile([128, NB], F32, tag="lt")
    nc.vector.tensor_sub(lt, l_den, l_num)

    lp = sb.tile([128, 1], F32, tag="lp")
    nc.vector.tensor_reduce(out=lp, in_=lt, axis=AX.X, op=ALU.add)
    lp_all = sb.tile([128, 1], F32, tag="lp_all")
    nc.gpsimd.partition_all_reduce(
        lp_all, lp, channels=128, reduce_op=bass_isa.ReduceOp.add
    )
    nc.sync.dma_start(out=outp[:, :], in_=lp_all[0:1, :])


def build_nc():
    nc = bacc.Bacc()
    feat = nc.declare_dram_parameter("features", [C, N], F32, isOutput=False)
    lab = nc.declare_dram_parameter("labels", [128, NB], F32, isOutput=False)
    outp = nc.declare_dram_parameter("out", [1, 1], F32, isOutput=True)
    with tile.TileContext(nc) as tc:
        with ExitStack() as ctx:
            _body(ctx, tc, feat[:, :], lab[:, :], outp)
    nc.finalize()
    return nc


_NC_CACHE = None


def _get_nc():
    global _NC_CACHE
    if _NC_CACHE is None:
        _NC_CACHE = build_nc()
    return _NC_CACHE


def make_in_maps(features: np.ndarray, labels_all: np.ndarray):
    in_maps = []
    for i in range(B):
        f = np.ascontiguousarray(features[i], dtype=np.float32)
        # labels_sb[p, b] = labels[128*b + p]
        l = np.ascontiguousarray(
            labels_all[i].astype(np.float32).reshape(NB, 128).T
        )
        in_maps.append({"features": f, "labels": l})
    return in_maps


def kernel(features: np.ndarray, labels_all: np.ndarray) -> np.ndarray:
    nc = _get_nc()
    in_maps = make_in_maps(features, labels_all)
    r = run_bass_kernel_spmd(nc, in_maps, core_ids=list(range(B)))
    sums = np.array([r.results[i]["out"][0, 0] for i in range(B)], dtype=np.float64)
    return np.float32(np.mean(sums) / N)


# revision 25
# speedup vs baseline: 1.0352x; 1.0024x over previous
"""Supervised-contrastive point-cloud loss on 8 TRN2 NeuronCores.

Full inputs: features [8, 128, 4096] f32, labels_all [8, 4096] int.
Data-parallel: one cloud per core. Each core computes the SUM of per-point
losses for its cloud; the host averages (sum / N / B).

Math (per cloud, fmap [C=128, N=4096], labels [N], 16 classes):
  v = normalize(fmap.T)                 (rows unit-norm)
  E = exp(v @ v.T)                      (TEMP cancels in pos/(pos+neg))
  sel[i] = sum_{j: lab j == lab i} E[ij]   (incl. diagonal e)
  T[i]   = sum_j E[ij]
  A = sel - e ; B = T - sel ; n = count[lab_i] ; nbar = N - n
  loss_i = ln(A*nbar + B*n) - ln(A*nbar)
The NxN mask is never materialized: per-class sums come from a one-hot
[128,16] matmul against exp tiles, accumulated in PSUM over j-blocks.

Engine budget per core: ACT does the 16.8M exps (the wall), PE does the gram
+ class-sum matmuls, DVE does normalize/epilogue elementwise, GPSIMD does
partition reductions (norms, counts, final sum), DMA does partition
broadcasts. PSUM: rotating "work" pool ([128,1024] x 3 = 6 banks) for gram
tiles + the transposed-class-sum accumulator, plus two rotating [16,512]
class-sum accumulators (2 banks). Everything else is PSUM-free, which keeps
every matmul at <=2 sync waits (walrus MM limit; bacc moves extras onto
ldweights).
"""

import numpy as np
from contextlib import ExitStack

import concourse.bass as bass
import concourse.bacc as bacc
import concourse.bass_isa as bass_isa
import concourse.tile as tile
from concourse import mybir
from concourse.bass_utils import run_bass_kernel_spmd

F32 = mybir.dt.float32
BF16 = mybir.dt.bfloat16
I32 = mybir.dt.int32
AF = mybir.ActivationFunctionType
ALU = mybir.AluOpType
AX = mybir.AxisListType

B = 8
C = 128
N = 4096
NB = N // 128          # 32 point blocks of 128
NCLS = 16
ISUP = 1024            # i-super width (columns of E per cs accumulator)
NSUP = N // ISUP       # 4
HW = 1024              # work-tile / exp chunk width (2 PSUM banks)
E_CONST = float(np.exp(1.0))


def _body(ctx: ExitStack, tc: "tile.TileContext", feat, lab, outp):
    nc = tc.nc

    const = ctx.enter_context(tc.tile_pool(name="const", bufs=1))
    sb = ctx.enter_context(tc.tile_pool(name="sb", bufs=1))
    e_pool = ctx.enter_context(tc.tile_pool(name="e", bufs=8))
    work = ctx.enter_context(tc.tile_pool(name="work", bufs=3, space="PSUM"))
    csp = ctx.enter_context(tc.tile_pool(name="csp", bufs=2, space="PSUM"))

    def wtile(shape, tag="work"):
        return work.tile(shape, F32, tag=tag, name="wk")

    # Preload the one ACT table set that serves every function we use
    # (natural_log_exp_and_others: exp, ln, copy, identity) so the bacc
    # fixpoint pass doesn't insert per-function loads mid-kernel.
    from concourse.hw_specs import get_activation_tables

    tables = list(get_activation_tables(nc.m.arch).keys())
    nle_id = tables.index("natural_log_exp_and_others")
    tl = mybir.InstLoadActFuncSet(
        name=nc.get_next_instruction_name(), act_func_set_id=nle_id, ins=[], outs=[]
    )
    nc.scalar.add_instruction(tl)

    # ---------------- load + normalize features (chunk-pipelined) ----------
    # per 1024-col chunk: DMA -> vsq (DVE) -> ns partition-reduce (GPSIMD) ->
    # ln (ACT) -> rinv = exp(-0.5*ln) (ACT) -> partition-broadcast (DMA) ->
    # vn = v * rinv_bc (DVE, bf16 out). No PSUM anywhere: the main loop's
    # gram tiles own the work pool from t=0. (Rsqrt ACT table is banned.)
    v_sb = sb.tile([128, N], F32, tag="v_sb")
    vsq = sb.tile([128, N], F32, tag="vsq")
    ns_all = sb.tile([128, N], F32, tag="ns_all")
    lns = sb.tile([128, N], F32, tag="lns")
    rinv_bc = sb.tile([128, N], BF16, tag="rinv_bc")
    vn_bf = sb.tile([128, N], BF16, tag="vn_bf")
    bounds = [0, 512, 1024, 2048, 4096]
    for cl, ch in zip(bounds[:-1], bounds[1:]):
        nc.sync.dma_start(out=v_sb[:, cl:ch], in_=feat[:, cl:ch])
        nc.vector.tensor_mul(vsq[:, cl:ch], v_sb[:, cl:ch], v_sb[:, cl:ch])
        nc.gpsimd.partition_all_reduce(
            ns_all[:, cl:ch], vsq[:, cl:ch], channels=128,
            reduce_op=bass_isa.ReduceOp.add,
        )
        nc.scalar.activation(lns[:, cl:ch], ns_all[:, cl:ch], AF.Ln)
        nc.scalar.activation(rinv_bc[:, cl:ch], lns[:, cl:ch], AF.Exp, scale=-0.5)
        nc.vector.tensor_mul(vn_bf[:, cl:ch], v_sb[:, cl:ch], rinv_bc[:, cl:ch])

    # ---------------- constants ----------------
    iota_i = const.tile([128, NCLS], I32, tag="iota_i")
    nc.gpsimd.iota(iota_i, pattern=[[1, NCLS]], base=0, channel_multiplier=0)
    iota_f = const.tile([128, NCLS], F32, tag="iota_f")
    nc.vector.tensor_copy(iota_f, iota_i)

    pidx_i = const.tile([128, 1], I32, tag="pidx_i")
    nc.gpsimd.iota(pidx_i, pattern=[[1, 1]], base=0, channel_multiplier=1)
    pidx_f = const.tile([128, 1], F32, tag="pidx_f")
    nc.vector.tensor_copy(pidx_f, pidx_i)

    i128 = const.tile([128, 128], I32, tag="i128")
    nc.gpsimd.iota(i128, pattern=[[1, 128]], base=0, channel_multiplier=0)
    i128_f = const.tile([128, 128], F32, tag="i128_f")
    nc.vector.tensor_copy(i128_f, i128)
    ident128 = const.tile([128, 128], F32, tag="ident128")
    nc.vector.tensor_scalar(
        out=ident128, in0=i128_f, scalar1=pidx_f, scalar2=None, op0=ALU.is_equal
    )

    # ---------------- labels -> one-hot + class counts (PSUM-free) ---------
    labels_sb = sb.tile([128, NB], F32, tag="labels_sb")
    nc.gpsimd.dma_start(out=labels_sb, in_=lab[:, :])

    oh_f = sb.tile([128, NB * NCLS], F32, tag="oh_f")  # [128, 512]
    for b in range(NB):
        nc.vector.tensor_scalar(
            out=oh_f[:, b * NCLS : (b + 1) * NCLS],
            in0=iota_f,
            scalar1=labels_sb[:, b : b + 1],
            scalar2=None,
            op0=ALU.is_equal,
        )
    oh_b = sb.tile([128, NB * NCLS], BF16, tag="oh_b")
    nc.vector.tensor_copy(oh_b, oh_f)

    # counts[c] = #points of class c: partition all-reduce then fold blocks
    cnt_all = sb.tile([128, NB * NCLS], F32, tag="cnt_all")
    nc.gpsimd.partition_all_reduce(
        cnt_all, oh_f, channels=128, reduce_op=bass_isa.ReduceOp.add
    )
    n_bc = sb.tile([128, NCLS], F32, tag="n_bc")
    nc.vector.tensor_reduce(
        out=n_bc,
        in_=cnt_all.rearrange("p (b c) -> p c b", c=NCLS),
        axis=AX.X,
        op=ALU.add,
    )

    n_rep = sb.tile([128, NB * NCLS], F32, tag="n_rep")
    for b in range(NB):
        nc.vector.tensor_copy(n_rep[:, b * NCLS : (b + 1) * NCLS], n_bc)
    # n_row[p, b] = count[label of point 128*b+p]
    n_row = sb.tile([128, NB], F32, tag="n_row")
    nrm = sb.tile([128, NB * NCLS], F32, tag="nrm")
    nc.vector.tensor_mul(nrm, oh_f, n_rep)
    nc.vector.tensor_reduce(
        out=n_row,
        in_=nrm.rearrange("p (b c) -> p b c", c=NCLS),
        axis=AX.X,
        op=ALU.add,
    )

    # ---------------- main loop: gram -> exp -> class-sum ------------------
    # cst_ps accumulates the transposed class-sums; allocated at the first
    # super's epilogue, held to the end (work rotation drops 3 -> 2 slots).
    cst_ps = wtile([128, NB * NCLS])  # [128, 512], held to the end
    masked = sb.tile([128, NB * NCLS], F32, tag="masked")

    sel = sb.tile([128, NB], F32, tag="sel")
    tot = sb.tile([128, NB], F32, tag="tot")

    def emit_super_epilogue(s, cs_half):
        # evacuate + transpose a finished super's class-sums; emitted a few
        # j-iterations into the NEXT super so the PE's in-order queue doesn't
        # stall its gram stream on the DVE evacuation copies. The last super's
        # first copy runs on ACT (idle by then) in parallel with DVE.
        cs_sb = sb.tile([NCLS, ISUP], F32, tag=f"cs_sb{s}", name=f"cs_sb{s}")
        nblk = ISUP // 128
        for hh in range(2):
            dst = cs_sb[:, hh * (ISUP // 2) : (hh + 1) * (ISUP // 2)]
            if s == NSUP - 1 and hh == 0:
                nc.scalar.copy(dst, cs_half[hh])
            else:
                nc.vector.tensor_copy(dst, cs_half[hh])
            for bb in range(hh * nblk // 2, (hh + 1) * nblk // 2):
                b = s * nblk + bb
                nc.tensor.transpose(
                    cst_ps[:, b * NCLS : (b + 1) * NCLS],
                    in_=cs_sb[:, bb * 128 : (bb + 1) * 128],
                    identity=ident128[0:NCLS, 0:NCLS],
                )
        lo, hi = s * nblk * NCLS, (s + 1) * nblk * NCLS
        nc.vector.tensor_mul(
            masked[:, lo:hi], cst_ps[:, lo:hi], oh_f[:, lo:hi]
        )
        nc.vector.tensor_reduce(
            out=sel[:, s * nblk : (s + 1) * nblk],
            in_=masked[:, lo:hi].rearrange("p (b c) -> p b c", c=NCLS),
            axis=AX.X,
            op=ALU.add,
        )
        nc.vector.tensor_reduce(
            out=tot[:, s * nblk : (s + 1) * nblk],
            in_=cst_ps[:, lo:hi].rearrange("p (b c) -> p b c", c=NCLS),
            axis=AX.X,
            op=ALU.add,
        )

    pending = None  # (s, cs_half) of the previous super, epilogue not emitted
    for s in range(NSUP):
        # two 1-bank accumulators: the next super's WAR waits only its half's
        # evacuation copy instead of the full 2-bank one
        cs_half = [
            csp.tile([NCLS, ISUP // 2], F32, tag="cs", name=f"cs{s}_{q}")
            for q in range(2)
        ]
        for j in range(NB):
            if j == 8 and pending is not None:
                emit_super_epilogue(*pending)
                pending = None
            for h in range(ISUP // HW):
                g = wtile([128, HW])
                for q in range(HW // 512):
                    col = s * ISUP + h * HW + q * 512
                    nc.tensor.matmul(
                        g[:, q * 512 : (q + 1) * 512],
                        lhsT=vn_bf[:, j * 128 : (j + 1) * 128],
                        rhs=vn_bf[:, col : col + 512],
                        start=True,
                        stop=True,
                    )
                e = e_pool.tile([128, HW], BF16, tag="e")
                nc.scalar.activation(e, g, AF.Exp)
                for q in range(HW // 512):
                    nc.tensor.matmul(
                        cs_half[q],
                        lhsT=oh_b[:, j * NCLS : (j + 1) * NCLS],
                        rhs=e[:, q * 512 : (q + 1) * 512],
                        start=(j == 0),
                        stop=(j == NB - 1),
                    )
        pending = (s, cs_half)
    emit_super_epilogue(*pending)

    # ---------------- epilogue ----------------
    a_t = sb.tile([128, NB], F32, tag="a_t")
    nc.vector.tensor_scalar_add(a_t, sel, -E_CONST)  # A = sel - e
    b_t = sb.tile([128, NB], F32, tag="b_t")
    nc.vector.tensor_sub(b_t, tot, sel)  # B = T - sel
    nbar = sb.tile([128, NB], F32, tag="nbar")
    nc.vector.tensor_scalar(
        out=nbar, in0=n_row, scalar1=-1.0, scalar2=float(N),
        op0=ALU.mult, op1=ALU.add,
    )
    num = sb.tile([128, NB], F32, tag="num")
    nc.vector.tensor_mul(num, a_t, nbar)
    den = sb.tile([128, NB], F32, tag="den")
    nc.vector.tensor_mul(den, b_t, n_row)
    nc.vector.tensor_add(den, den, num)

    l_den = sb.tile([128, NB], F32, tag="l_den")
    nc.scalar.activation(l_den, den, AF.Ln)
    l_num = sb.tile([128, NB], F32, tag="l_num")
    nc.scalar.activation(l_num, num, AF.Ln)
    lt = sb.tile([128, NB], F32, tag="lt")
    nc.vector.tensor_sub(lt, l_den, l_num)

    lp = sb.tile([128, 1], F32, tag="lp")
    nc.vector.tensor_reduce(out=lp, in_=lt, axis=AX.X, op=ALU.add)
    lp_all = sb.tile([128, 1], F32, tag="lp_all")
    nc.gpsimd.partition_all_reduce(
        lp_all, lp, channels=128, reduce_op=bass_isa.ReduceOp.add
    )
    nc.sync.dma_start(out=outp[:, :], in_=lp_all[0:1, :])


def build_nc():
    nc = bacc.Bacc()
    feat = nc.declare_dram_parameter("features", [C, N], F32, isOutput=False)
    lab = nc.declare_dram_parameter("labels", [128, NB], F32, isOutput=False)
    outp = nc.declare_dram_parameter("out", [1, 1], F32, isOutput=True)
    with tile.TileContext(nc) as tc:
        with ExitStack() as ctx:
            _body(ctx, tc, feat[:, :], lab[:, :], outp)
    nc.finalize()
    return nc


_NC_CACHE = None


def _get_nc():
    global _NC_CACHE
    if _NC_CACHE is None:
        _NC_CACHE = build_nc()
    return _NC_CACHE


def make_in_maps(features: np.ndarray, labels_all: np.ndarray):
    in_maps = []
    for i in range(B):
        f = np.ascontiguousarray(features[i], dtype=np.float32)
        # labels_sb[p, b] = labels[128*b + p]
        l = np.ascontiguousarray(
            labels_all[i].astype(np.float32).reshape(NB, 128).T
        )
        in_maps.append({"features": f, "labels": l})
    return in_maps


def kernel(features: np.ndarray, labels_all: np.ndarray) -> np.ndarray:
    nc = _get_nc()
    in_maps = make_in_maps(features, labels_all)
    r = run_bass_kernel_spmd(nc, in_maps, core_ids=list(range(B)))
    sums = np.array([r.results[i]["out"][0, 0] for i in range(B)], dtype=np.float64)
    return np.float32(np.mean(sums) / N)
